# revision 1
# baseline (speedup 1.0000x reference)
"""RWKV WKV recurrence kernel for Trainium2 (8 NeuronCores).

Math: for each (batch, channel) pair, over time t:
    num_t = a_{t-1} + e^{u+k_t} * v_t
    den_t = b_{t-1} + e^{u+k_t}
    out_t = num_t / den_t
    a_t   = e^w * a_{t-1} + e^{k_t} * v_t
    b_t   = e^w * b_{t-1} + e^{k_t}
with w = -exp(time_decay) < 0, u = time_first. The reference uses a
log-sum-exp-stabilized form of the same recurrence; for these inputs
(k ~ N(0,1), strictly negative decay) the state is geometrically bounded
(|a|,b < ~20) so the direct fp32 form matches to ~1e-7 absmax.

Mapping: batch (8) -> one NeuronCore each. Per core, channels go on SBUF
partitions (16 groups of 128) and time along the free dimension, so the
whole T=2048 recurrence per group is ONE DVE tensor_tensor_scan
(state = ew*state + in, fp32 internal). Inputs arrive [T, H] row-major, so
[128t x 128h] chunks are PE-transposed (via identity matmul) into
channel-major PSUM banks; exp() runs on ScalarE straight out of PSUM.
"""

import os
import sys
from contextlib import ExitStack

import numpy as np

for _p in ("/opt/trn_rl_repo", "/root/.axon_site/_ro/trn_rl_repo"):
    if os.path.isdir(_p) and _p not in sys.path:
        sys.path.insert(0, _p)

import concourse.bacc as bacc
import concourse.mybir as mybir
import concourse.tile as tile
from concourse import dve_ops as _dve_ops
from concourse import dve_spec as _dve_spec
from concourse import masks
from concourse.bass_utils import run_bass_kernel_spmd
from concourse.dve_spec import Spec as _Spec, lower as _dve_lower
from concourse.dve_table_gen import dve_ver_for as _dve_ver_for
from concourse.dve_uop import AluOp as _AluOp, DveOpSpec as _DveOpSpec

F32 = mybir.dt.float32
AF = mybir.ActivationFunctionType
OP = mybir.AluOpType

B, T, H = 8, 2048, 2048
N_CORES = 8

# Fused out = num/den in ONE DVE pass: bitwise-NOT reciprocal seed
# (x*bitcast(~x) lands in [-4.5,-4]) + minimax deg-2 polynomial + multiply.
# Relative error ~5.1e-5 (vs ~51 ULP for fast+NR at 3 passes).
_DIV_C0, _DIV_C1, _DIV_C2 = -0.7071067, -0.1665221, -0.013060556


def _div_mul_ref(in0, in1, c0, c1, c2):
    in0 = np.asarray(in0, np.float32)
    in1 = np.asarray(in1, np.float32)
    n = (~in0.view(np.int32)).view(np.float32)
    s = (in0 * n).astype(np.float32)
    q = (in1 * n).astype(np.float32)
    u = (s * np.float32(c2)).astype(np.float32)
    v = (np.float32(c1) + u).astype(np.float32)
    w = (s * v).astype(np.float32)
    p = (np.float32(c0) + w).astype(np.float32)
    return (q * p).astype(np.float32)


def _register_div_mul():
    name = "WKV_DIV_MUL_ANT"
    if name in _dve_ops._SUB_OPCODE_FOR_NAME:
        return next(o for o in _dve_ops.OPS if o.name == name)
    Src0, Src1 = _dve_spec.Src0, _dve_spec.Src1
    C0, C1, C2 = _dve_spec.C0, _dve_spec.C1, _dve_spec.C2
    _n = _dve_spec.Bin(_AluOp.BITWISE_NOT, Src0, Src0)
    _s = Src0 * _n
    body = (Src1 * _n) * (C0 + _s * (C1 + _s * C2))
    spec = _Spec(body=body, reference=_div_mul_ref)
    shas = {}
    for ver in ("v3", "v4"):
        try:
            uops = _dve_lower(spec, ver=ver)
        except Exception:
            continue
        shas[ver] = _DveOpSpec(name=name, opcode=0, uops=uops, rd1_en=True).sha(ver)
    op = _dve_ops.DveOp(name, spec, subdim=False, uops_sha=shas)
    row = _dve_ops._CUSTOM_DVE_ROW_BASE + len(_dve_ops.OPS)
    assert row < 0x20
    _dve_ops.OPS.append(op)
    _dve_ops._SUB_OPCODE_FOR_NAME[name] = row
    _dve_ops.CUSTOM_DVE_SPECS[name] = spec
    return op


WKV_DIV_MUL = _register_div_mul()


def build_nc(t=T, h=H, recip_mode="div2", repeat=1, ablate=()):
    """Build the single-core program (SPMD across cores via differing inputs).

    repeat>1 duplicates the whole compute loop (same outputs) — used only to
    measure device time as the slope of wall time vs repeat count.
    ablate: set of stage names to skip (timing experiments only; output wrong):
      {"scan", "recip", "stt", "ekv", "outmul", "outpath", "inpath"}
    """
    ab = set(ablate)
    nc = bacc.Bacc("TRN2", target_bir_lowering=False, debug=False)

    key = nc.dram_tensor("key", [t, h], F32, kind="ExternalInput").ap()
    value = nc.dram_tensor("value", [t, h], F32, kind="ExternalInput").ap()
    td = nc.dram_tensor("time_decay", [h], F32, kind="ExternalInput").ap()
    tf = nc.dram_tensor("time_first", [h], F32, kind="ExternalInput").ap()
    out = nc.dram_tensor("out", [t, h], F32, kind="ExternalOutput").ap()

    G = h // 128  # channel groups (partition dim)
    S = t // 128  # 128-wide time chunks
    SB = min(512, t)  # PSUM bank tile free width (512 f32 = 1 bank)
    CPB = SB // 128  # time chunks per PSUM bank tile
    NB = S // CPB  # bank tiles per group

    with tile.TileContext(nc) as tc, ExitStack() as ctx:
        const = ctx.enter_context(tc.tile_pool(name="const", bufs=1))
        identity = const.tile([128, 128], F32)
        masks.make_identity(nc, identity[:])

        tf_t = const.tile([128, G], F32)
        nc.sync.dma_start(tf_t[:], tf.rearrange("(g p) -> p g", p=128))
        td_t = const.tile([128, G], F32)
        nc.sync.dma_start(td_t[:], td.rearrange("(g p) -> p g", p=128))
        eu_t = const.tile([128, G], F32)
        nc.scalar.activation(eu_t[:], tf_t[:], AF.Exp)
        etd_t = const.tile([128, G], F32)
        nc.scalar.activation(etd_t[:], td_t[:], AF.Exp)
        ew_t = const.tile([128, G], F32)  # e^w = exp(-exp(td))
        nc.scalar.activation(ew_t[:], etd_t[:], AF.Exp, scale=-1.0)

        chunks = ctx.enter_context(tc.tile_pool(name="chunks", bufs=2))
        psum_in = ctx.enter_context(tc.tile_pool(name="psum_in", bufs=2, space="PSUM"))
        psum_out = ctx.enter_context(
            tc.tile_pool(name="psum_out", bufs=2, space="PSUM")
        )
        grp = ctx.enter_context(tc.tile_pool(name="grp", bufs=2))
        stage = ctx.enter_context(tc.tile_pool(name="stage", bufs=2))

        for g in [gg for _ in range(repeat) for gg in range(G)]:
            eu_g = eu_t[:, g : g + 1]
            ew_g = ew_t[:, g : g + 1]
            hs = slice(g * 128, (g + 1) * 128)

            ek = grp.tile([128, t], F32, tag="ek")
            ekv = grp.tile([128, t], F32, tag="ekv")
            A = grp.tile([128, t + 1], F32, tag="A")
            Bb = grp.tile([128, t + 1], F32, tag="B")
            num = grp.tile([128, t], F32, tag="num")
            den = grp.tile([128, t], F32, tag="den")
            rcp = None
            if recip_mode != "div2":
                rcp = grp.tile([128, t], F32, tag="rcp")
            outg = grp.tile([128, t], F32, tag="outg")

            # ---- load (one 3D-AP DMA per tensor) + PE transpose; exp on ACT ----
            if "inpath" not in ab:
                kc_all = chunks.tile([128, t], F32, tag="kc")
                nc.sync.dma_start(
                    kc_all[:].rearrange("p (s h) -> p s h", h=128),
                    key[:, hs].rearrange("(s p) h -> p s h", p=128),
                )
                vc_all = chunks.tile([128, t], F32, tag="vc")
                nc.scalar.dma_start(
                    vc_all[:].rearrange("p (s h) -> p s h", h=128),
                    value[:, hs].rearrange("(s p) h -> p s h", p=128),
                )
                for nb in range(NB):
                    kT = psum_in.tile([128, SB], F32, tag="kT")
                    vT = psum_in.tile([128, SB], F32, tag="vT")
                    for c in range(CPB):
                        s = nb * CPB + c
                        ts_ = slice(s * 128, (s + 1) * 128)
                        nc.tensor.transpose(
                            kT[:, c * 128 : (c + 1) * 128], kc_all[:, ts_], identity[:]
                        )
                        nc.tensor.transpose(
                            vT[:, c * 128 : (c + 1) * 128], vc_all[:, ts_], identity[:]
                        )
                    bsl = slice(nb * SB, (nb + 1) * SB)
                    nc.scalar.activation(ek[:, bsl], kT[:], AF.Exp)
                    if "ekv" not in ab:
                        nc.vector.tensor_mul(ekv[:, bsl], ek[:, bsl], vT[:])

            # ---- the recurrence: one scan per group, fp32 state ----
            src = ekv if "ekv" not in ab else ek  # always-written scan input
            d0 = ew_g.broadcast_to((128, t))
            if "scan" not in ab:
                nc.vector.memset(A[:, 0:1], 0.0)
                nc.vector.memset(Bb[:, 0:1], 0.0)
                nc.vector.tensor_tensor_scan(
                    A[:, 1 : t + 1], d0, src[:], 0.0, OP.mult, OP.add
                )
                nc.vector.tensor_tensor_scan(
                    Bb[:, 1 : t + 1], d0, ek[:], 0.0, OP.mult, OP.add
                )
                A_r, B_r = A[:, 0:t], Bb[:, 0:t]
            else:
                A_r, B_r = src[:], ek[:]
            # num = eu*ekv + a_{t-1}; den = eu*ek + b_{t-1}
            if "stt" not in ab:
                nc.vector.scalar_tensor_tensor(
                    num[:], src[:], eu_g, A_r, OP.mult, OP.add
                )
                nc.vector.scalar_tensor_tensor(
                    den[:], ek[:], eu_g, B_r, OP.mult, OP.add
                )
                num_r, den_r = num, den
            else:
                num_r, den_r = src, ek
            if "recip" in ab or "outmul" in ab:
                outg_r = num_r
            elif recip_mode == "div2":
                # out = num/den in one fused DVE pass
                nc.vector._custom_dve(
                    WKV_DIV_MUL,
                    out=outg[:],
                    in0=den_r[:],
                    in1=num_r[:],
                    s0=_DIV_C0,
                    s1=_DIV_C1,
                    imm2=_DIV_C2,
                )
                outg_r = outg
            else:
                if recip_mode == "accurate":
                    # scratch: ekv is fully consumed by this point
                    nc.vector.reciprocal_approx_accurate(rcp[:], den_r[:], ekv[:])
                elif recip_mode == "fast":
                    nc.vector.reciprocal_approx_fast(rcp[:], den_r[:])
                else:
                    nc.vector.reciprocal(rcp[:], den_r[:])
                nc.vector.tensor_mul(outg[:], num_r[:], rcp[:])
                outg_r = outg

            # ---- transpose back to [T, H]; one 3D-AP store per group ----
            if "outpath" not in ab:
                ost = stage.tile([128, t], F32, tag="ost")
                for nb in range(NB):
                    oT = psum_out.tile([128, SB], F32, tag="oT")
                    for c in range(CPB):
                        s = nb * CPB + c
                        nc.tensor.transpose(
                            oT[:, c * 128 : (c + 1) * 128],
                            outg_r[:, s * 128 : (s + 1) * 128],
                            identity[:],
                        )
                    nc.scalar.copy(ost[:, nb * SB : (nb + 1) * SB], oT[:])
                nc.sync.dma_start(
                    out[:, hs].rearrange("(s p) h -> p s h", p=128),
                    ost[:].rearrange("p (s h) -> p s h", h=128),
                )

    nc.compile()
    return nc


_nc_cache = {}


def _get_nc():
    if "nc" not in _nc_cache:
        _nc_cache["nc"] = build_nc()
    return _nc_cache["nc"]


def kernel_with_results(key, value, time_decay, time_first, trace=False):
    nc = _get_nc()
    key = np.ascontiguousarray(key, dtype=np.float32)
    value = np.ascontiguousarray(value, dtype=np.float32)
    time_decay = np.ascontiguousarray(time_decay, dtype=np.float32)
    time_first = np.ascontiguousarray(time_first, dtype=np.float32)
    in_maps = [
        {
            "key": key[i],
            "value": value[i],
            "time_decay": time_decay,
            "time_first": time_first,
        }
        for i in range(N_CORES)
    ]
    res = run_bass_kernel_spmd(nc, in_maps, list(range(N_CORES)), trace=trace)
    out = np.stack([res.results[i]["out"] for i in range(N_CORES)], axis=0)
    return out, res


def kernel(key, value, time_decay, time_first):
    out, _ = kernel_with_results(key, value, time_decay, time_first)
    return out

